# revision 3
# baseline (speedup 1.0000x reference)
"""Causal self-attention (B=2, N=2048, D=768, H=12) on 8 Trainium2 NeuronCores.

Sharding: data-parallel over batch (2) x tensor-parallel over head groups (4),
3 heads per core. Each core computes, for its (batch, head-group):
  GEMM1: kqT/qT (transposed) and v (natural) projections from xT,
  scores^T = k @ q^T per head (f32r matmuls), exp on ScalarE,
  AV with a ones-augmented V (bf16) giving unnormalized sa + row sums,
  normalize, PE-transpose sa -> saT, GEMM2 row-parallel -> yT partial.
Host: shards inputs, sums the 4 per-batch partials (the "all-reduce"), adds
the output bias fold (bproj + bkqv_v @ Wproj works because softmax rows sum
to 1).

Self-contained: hardcodes all shapes; no sibling imports.
"""

import os

import numpy as np
import ml_dtypes

B, N, D = 2, 2048, 768
H, HD = 12, 64
HPC = 3           # heads per core
NG = 4            # head groups
NCORES = 8
P = 128
NJ = N // P       # 16 j-chunks (keys) per head
NI = N // P       # 16 i-chunks (queries)

_compiled = None  # (nc,) cache so repeated kernel() calls reuse the NEFF
last_exec_time_ns = None
last_results = None


def _build():
    import concourse.bass as bass
    import concourse.tile as tile
    import concourse.mybir as mybir
    from concourse import bacc
    from concourse.masks import make_identity

    f32 = mybir.dt.float32
    f32r = mybir.dt.float32r
    bf16 = mybir.dt.bfloat16
    ADD = mybir.AluOpType.add
    MULT = mybir.AluOpType.mult
    EXP = mybir.ActivationFunctionType.Exp

    nc = bacc.Bacc(
        "TRN2", target_bir_lowering=False, debug=False, num_devices=NCORES
    )

    xT_d = nc.dram_tensor("xT", [D, N], f32r, kind="ExternalInput").ap()
    wkq_d = nc.dram_tensor("wkq", [D, 384], f32r, kind="ExternalInput").ap()
    wv_d = nc.dram_tensor("wv", [D, 256], f32r, kind="ExternalInput").ap()
    wp_d = nc.dram_tensor("wp", [64, HPC, D], f32r, kind="ExternalInput").ap()
    bkq_d = nc.dram_tensor("bkq", [P, 4], f32, kind="ExternalInput").ap()
    mask_d = nc.dram_tensor("mask", [P, P], bf16, kind="ExternalInput").ap()
    yT_d = nc.dram_tensor("yT", [D, N], f32, kind="ExternalOutput").ap()

    xT_v = xT_d.rearrange("(po pi) f -> pi po f", pi=P)    # [128, 6, 2048]
    wkq_v = wkq_d.rearrange("(po pi) f -> pi po f", pi=P)  # [128, 6, 384]
    wv_v = wv_d.rearrange("(po pi) f -> pi po f", pi=P)    # [128, 6, 256]
    yT_v = yT_d.rearrange("(po pi) f -> pi po f", pi=P)    # [128, 6, 2048]

    with tile.TileContext(nc) as tc:
        import contextlib

        ctx = contextlib.ExitStack()
        with ctx:
            const = ctx.enter_context(tc.tile_pool(name="const", bufs=1))
            big = ctx.enter_context(tc.tile_pool(name="bigbufs", bufs=1))
            work = ctx.enter_context(tc.tile_pool(name="work", bufs=3))
            ypool = ctx.enter_context(tc.tile_pool(name="ypool", bufs=3))
            psum_big = ctx.enter_context(
                tc.tile_pool(name="psum_big", bufs=2, space="PSUM")
            )
            psum_sa = ctx.enter_context(
                tc.tile_pool(name="psum_sa", bufs=2, space="PSUM")
            )
            psum_tr = ctx.enter_context(
                tc.tile_pool(name="psum_tr", bufs=2, space="PSUM")
            )

            # ---- constants / weights to SBUF ----
            mask_t = const.tile([P, P], bf16, name="mask_t")
            nc.sync.dma_start(mask_t[:], mask_d)
            ident = const.tile([P, P], f32, name="ident")
            make_identity(nc, ident[:])
            wkq_t = const.tile([P, 6, 384], f32r, name="wkq_t")
            nc.sync.dma_start(wkq_t[:], wkq_v)
            wv_t = const.tile([P, 6, 256], f32r, name="wv_t")
            nc.sync.dma_start(wv_t[:], wv_v)
            wp_t = const.tile([64, HPC, D], f32r, name="wp_t")
            nc.sync.dma_start(wp_t[:], wp_d)
            bkq_t = const.tile([P, 4], f32, name="bkq_t")
            nc.sync.dma_start(bkq_t[:], bkq_d)
            xT_t = big.tile([P, 6, N], f32r, name="xT_t")
            nc.sync.dma_start(xT_t[:], xT_v)

            kT = big.tile([P, 2, N], f32r, name="kT")
            qT = big.tile([P, 2, N], f32r, name="qT")
            vaug = big.tile([P, NJ, HPC, 65], bf16, name="vaug")
            saT = big.tile([64, HPC, N], f32r, name="saT")

            def mm(out, lhsT, rhs, start, stop):
                nc.tensor.matmul(out, lhsT, rhs, start=start, stop=stop)

            # ---- GEMM1-kq: kqT chunks ----
            # wkq cols: [k01 (128) | q01 (128) | k2 (64) | q2 (64)]
            chunks = [
                (kT, 0, 128, 0, 128),
                (qT, 128, 256, 0, 128),
                (kT, 256, 320, 1, 64),
                (qT, 320, 384, 1, 64),
            ]
            for ci, (dst, lo, hi, cchunk, M) in enumerate(chunks):
                for isl in range(4):
                    ps = psum_big.tile([P, 1024], f32, tag="big", name="ps_kq")
                    for dc in range(6):
                        mm(
                            ps[0:M, 0:512],
                            wkq_t[:, dc, lo:hi],
                            xT_t[:, dc, 512 * isl : 512 * isl + 512],
                            start=(dc == 0),
                            stop=(dc == 5),
                        )
                    nc.vector.tensor_scalar(
                        dst[0:M, cchunk, 512 * isl : 512 * isl + 512],
                        ps[0:M, 0:512],
                        bkq_t[0:M, ci : ci + 1],
                        None,
                        op0=ADD,
                    )

            # ---- GEMM1-v: v natural, bf16, ones column ----
            nc.vector.memset(vaug[:, :, :, 64:65], 1.0)
            for ic in range(NI):
                ps = psum_big.tile([P, 1024], f32, tag="big", name="ps_v")
                for dc in range(6):
                    mm(
                        ps[:, 0:256],
                        xT_t[:, dc, 128 * ic : 128 * ic + 128],
                        wv_t[:, dc, :],
                        start=(dc == 0),
                        stop=(dc == 5),
                    )
                for h in range(HPC):
                    nc.vector.tensor_copy(
                        out=vaug[:, ic, h, 0:64],
                        in_=ps[:, 64 * h : 64 * h + 64],
                    )

            # ---- attention per head ----
            for h in range(HPC):
                if h < 2:
                    cc, pb = 0, 64 * h
                else:
                    cc, pb = 1, 0
                strips = []
                for jc in range(NJ):
                    i0 = 128 * jc
                    W = N - i0
                    strip = work.tile(
                        [P, W], bf16, tag=f"expT{jc}", bufs=1, name=f"expT{jc}"
                    )
                    for s0 in range(0, W, 1024):
                        sw = min(1024, W - s0)
                        ps = psum_big.tile([P, 1024], f32, tag="big", name="ps_s")
                        for sub in range(0, sw, 512):
                            ssw = min(512, sw - sub)
                            mm(
                                ps[:, sub : sub + ssw],
                                kT[pb : pb + 64, cc, i0 : i0 + 128],
                                qT[pb : pb + 64, cc, i0 + s0 + sub : i0 + s0 + sub + ssw],
                                start=True,
                                stop=True,
                            )
                        nc.scalar.activation(
                            strip[:, s0 : s0 + sw],
                            ps[:, 0:sw],
                            EXP,
                            scale=0.125,
                        )
                    # causal mask on the diagonal 128-block
                    nc.vector.tensor_tensor(
                        strip[:, 0:128], strip[:, 0:128], mask_t[:], MULT
                    )
                    strips.append(strip)
                for ic in range(NI):
                    pa = psum_sa.tile([P, 128], f32, tag="sa", name="pa")
                    for jc in range(ic + 1):
                        off = 128 * (ic - jc)
                        nc.tensor.matmul(
                            pa[:, 0:65],
                            strips[jc][:, off : off + 128],
                            vaug[:, jc, h, :],
                            start=(jc == 0),
                            stop=(jc == ic),
                        )
                    recip = work.tile([P, 1], f32, tag="recip", name="recip")
                    nc.vector.reciprocal(recip[:], pa[:, 64:65])
                    sa_n = work.tile([P, 64], f32, tag="sa_n", name="sa_n")
                    nc.vector.tensor_scalar(
                        sa_n[:], pa[:, 0:64], recip[:, 0:1], None, op0=MULT
                    )
                    pt = psum_tr.tile([64, 128], f32, tag="tr", name="pt")
                    nc.tensor.transpose(pt[:], sa_n[:], ident[:])
                    nc.vector.tensor_copy(
                        out=saT[:, h, 128 * ic : 128 * ic + 128], in_=pt[:]
                    )

            # ---- GEMM2: yT = sum_h wp_h^T @ saT_h ----
            for oc in range(6):
                for isl in range(4):
                    ps = psum_big.tile([P, 1024], f32, tag="big", name="ps_y")
                    for h in range(HPC):
                        mm(
                            ps[:, 0:512],
                            wp_t[:, h, 128 * oc : 128 * oc + 128],
                            saT[:, h, 512 * isl : 512 * isl + 512],
                            start=(h == 0),
                            stop=(h == HPC - 1),
                        )
                    yst = ypool.tile([P, 512], f32, tag="yst", name="yst")
                    nc.vector.tensor_copy(out=yst[:], in_=ps[:, 0:512])
                    nc.sync.dma_start(
                        yT_v[:, oc, 512 * isl : 512 * isl + 512], yst[:]
                    )

    nc.compile()
    return nc


def _host_prep(x, Wkqv, bkqv, Wproj, bproj):
    bf16 = ml_dtypes.bfloat16
    Wk = Wkqv[:, 0:D]
    Wq = Wkqv[:, D : 2 * D]
    Wv = Wkqv[:, 2 * D : 3 * D]
    bk = bkqv[0:D]
    bq = bkqv[D : 2 * D]
    bv = bkqv[2 * D : 3 * D]
    out_bias = (bproj + bv @ Wproj).astype(np.float32)  # softmax rows sum to 1

    jmask = np.triu(np.ones((P, P), np.float32)).astype(bf16)  # mask[j,i]=1 if i>=j

    in_maps = []
    for b in range(B):
        xT = np.ascontiguousarray(x[b].T)
        for g in range(NG):
            hs = [HPC * g + i for i in range(HPC)]
            wk = [np.asarray(Wk[:, HD * h : HD * h + HD]) for h in hs]
            wq = [np.asarray(Wq[:, HD * h : HD * h + HD]) for h in hs]
            wv = [np.asarray(Wv[:, HD * h : HD * h + HD]) for h in hs]
            wkq = np.ascontiguousarray(
                np.concatenate([wk[0], wk[1], wq[0], wq[1], wk[2], wq[2]], axis=1)
            )
            wv_pad = np.ascontiguousarray(
                np.concatenate(wv + [np.zeros((D, 64), np.float32)], axis=1)
            )
            wp = np.ascontiguousarray(
                np.stack([Wproj[HD * h : HD * h + HD, :] for h in hs], axis=1)
            )
            bkq = np.zeros((P, 4), np.float32)
            bkq[:, 0] = np.concatenate(
                [bk[HD * hs[0] : HD * hs[0] + HD], bk[HD * hs[1] : HD * hs[1] + HD]]
            )
            bkq[:, 1] = np.concatenate(
                [bq[HD * hs[0] : HD * hs[0] + HD], bq[HD * hs[1] : HD * hs[1] + HD]]
            )
            bkq[0:64, 2] = bk[HD * hs[2] : HD * hs[2] + HD]
            bkq[0:64, 3] = bq[HD * hs[2] : HD * hs[2] + HD]
            in_maps.append(
                dict(xT=xT, wkq=wkq, wv=wv_pad, wp=wp, bkq=bkq, mask=jmask)
            )
    return in_maps, out_bias


def kernel(x, Wkqv, bkqv, Wproj, bproj):
    global _compiled, last_exec_time_ns, last_results
    import concourse.bass_utils as bass_utils

    x = np.asarray(x, np.float32)
    Wkqv = np.asarray(Wkqv, np.float32)
    bkqv = np.asarray(bkqv, np.float32)
    Wproj = np.asarray(Wproj, np.float32)
    bproj = np.asarray(bproj, np.float32)

    if _compiled is None:
        _compiled = _build()
    nc = _compiled

    in_maps, out_bias = _host_prep(x, Wkqv, bkqv, Wproj, bproj)

    trace = os.environ.get("BASS_KERNEL_TRACE", "0") == "1"
    res = bass_utils.run_bass_kernel_spmd(
        nc, in_maps, core_ids=list(range(NCORES)), trace=trace
    )
    last_exec_time_ns = res.exec_time_ns
    last_results = res

    out = np.zeros((B, N, D), np.float32)
    for b in range(B):
        acc = np.zeros((D, N), np.float32)
        for g in range(NG):
            acc += res.results[b * NG + g]["yT"]
        out[b] = acc.T + out_bias
    return out


# revision 4
# speedup vs baseline: 1.1538x; 1.1538x over previous
"""Causal self-attention (B=2, N=2048, D=768, H=12) on 8 Trainium2 NeuronCores.

Sharding: data-parallel over batch (2) x tensor-parallel over head groups (4),
3 heads per core. Each core computes, for its (batch, head-group):
  GEMM1: kT/qT (transposed) and v (natural) projections from xT,
  scores^T = k @ q^T per head, exp on ScalarE (fp16 out),
  AV with a ones-augmented V giving unnormalized sa + row sums,
  normalize, PE-transpose sa -> saT, GEMM2 row-parallel -> yT partial.
All matmul operands are fp16 (fp32 PSUM accumulate); measured end-to-end
scaled relative error ~4e-4. Host shards inputs, sums the 4 per-batch
partials (the "all-reduce"), and adds the output bias fold
(bproj + bkqv_v @ Wproj — exact because softmax rows sum to 1).

Self-contained: hardcodes all shapes; no sibling imports.
"""

import os

import numpy as np

B, N, D = 2, 2048, 768
H, HD = 12, 64
HPC = 3           # heads per core
NG = 4            # head groups
NCORES = 8
P = 128
NJ = N // P       # 16 j-chunks (keys) per head
NI = N // P       # 16 i-chunks (queries)

_compiled = None  # cached compiled Bass module
last_exec_time_ns = None
last_results = None


def _build():
    import concourse.tile as tile
    import concourse.mybir as mybir
    from concourse import bacc

    f32 = mybir.dt.float32
    f16 = mybir.dt.float16
    ADD = mybir.AluOpType.add
    MULT = mybir.AluOpType.mult
    EXP = mybir.ActivationFunctionType.Exp

    nc = bacc.Bacc(
        "TRN2", target_bir_lowering=False, debug=False, num_devices=NCORES
    )

    xT_d = nc.dram_tensor("xT", [D, N], f16, kind="ExternalInput").ap()
    wkq_d = nc.dram_tensor("wkq", [D, 384], f16, kind="ExternalInput").ap()
    wv_d = nc.dram_tensor("wv", [D, 192], f16, kind="ExternalInput").ap()
    wp_d = nc.dram_tensor("wp", [P, 2, D], f16, kind="ExternalInput").ap()
    bkq_d = nc.dram_tensor("bkq", [P, 4], f32, kind="ExternalInput").ap()
    mask_d = nc.dram_tensor("mask", [P, P], f16, kind="ExternalInput").ap()
    ident_d = nc.dram_tensor("ident", [P, P], f16, kind="ExternalInput").ap()
    yT_d = nc.dram_tensor("yT", [D, N], f32, kind="ExternalOutput").ap()

    xT_v = xT_d.rearrange("(po pi) f -> pi po f", pi=P)    # [128, 6, 2048]
    wkq_v = wkq_d.rearrange("(po pi) f -> pi po f", pi=P)  # [128, 6, 384]
    wv_v = wv_d.rearrange("(po pi) f -> pi po f", pi=P)    # [128, 6, 192]
    yT_v = yT_d.rearrange("(po pi) f -> pi po f", pi=P)    # [128, 6, 2048]

    with tile.TileContext(nc) as tc:
        import contextlib

        ctx = contextlib.ExitStack()
        with ctx:
            const = ctx.enter_context(tc.tile_pool(name="const", bufs=1))
            big = ctx.enter_context(tc.tile_pool(name="bigbufs", bufs=1))
            work = ctx.enter_context(tc.tile_pool(name="work", bufs=3))
            ypool = ctx.enter_context(tc.tile_pool(name="ypool", bufs=3))
            psum_big = ctx.enter_context(
                tc.tile_pool(name="psum_big", bufs=2, space="PSUM")
            )
            psum_sa = ctx.enter_context(
                tc.tile_pool(name="psum_sa", bufs=2, space="PSUM")
            )
            psum_tr = ctx.enter_context(
                tc.tile_pool(name="psum_tr", bufs=2, space="PSUM")
            )

            # ---- constants / weights to SBUF ----
            mask_t = const.tile([P, P], f16, name="mask_t")
            nc.sync.dma_start(mask_t[:], mask_d)
            ident = const.tile([P, P], f16, name="ident")
            nc.sync.dma_start(ident[:], ident_d)
            wkq_t = const.tile([P, 6, 384], f16, name="wkq_t")
            nc.sync.dma_start(wkq_t[:], wkq_v)
            wv_t = const.tile([P, 6, 192], f16, name="wv_t")
            nc.sync.dma_start(wv_t[:], wv_v)
            wp_t = const.tile([P, 2, D], f16, name="wp_t")
            nc.sync.dma_start(wp_t[:], wp_d)
            bkq_t = const.tile([P, 4], f32, name="bkq_t")
            nc.sync.dma_start(bkq_t[:], bkq_d)
            xT_t = big.tile([P, 6, N], f16, name="xT_t")
            nc.sync.dma_start(xT_t[:], xT_v)

            kT = big.tile([P, 2, N], f16, name="kT")
            qT = big.tile([P, 2, N], f16, name="qT")
            vaug = big.tile([P, NJ, HPC, 65], f16, name="vaug")
            saT = big.tile([P, 2, N], f16, name="saT")

            # ---- GEMM1-kq: kT/qT chunks ----
            # wkq cols: [k01 (128) | q01 (128) | k2 (64) | q2 (64)]
            chunks = [
                (kT, 0, 128, 0, 128),
                (qT, 128, 256, 0, 128),
                (kT, 256, 320, 1, 64),
                (qT, 320, 384, 1, 64),
            ]
            for ci, (dst, lo, hi, cchunk, M) in enumerate(chunks):
                for isl in range(4):
                    ps = psum_big.tile([P, 1024], f32, tag="big", name="ps_kq")
                    for dc in range(6):
                        nc.tensor.matmul(
                            ps[0:M, 0:512],
                            wkq_t[:, dc, lo:hi],
                            xT_t[:, dc, 512 * isl : 512 * isl + 512],
                            start=(dc == 0),
                            stop=(dc == 5),
                        )
                    nc.vector.tensor_scalar(
                        dst[0:M, cchunk, 512 * isl : 512 * isl + 512],
                        ps[0:M, 0:512],
                        bkq_t[0:M, ci : ci + 1],
                        None,
                        op0=ADD,
                    )

            # ---- GEMM1-v: v natural, ones column ----
            nc.vector.memset(vaug[:, :, :, 64:65], 1.0)
            for ic in range(NI):
                ps = psum_big.tile([P, 1024], f32, tag="big", name="ps_v")
                for dc in range(6):
                    nc.tensor.matmul(
                        ps[:, 0:192],
                        xT_t[:, dc, 128 * ic : 128 * ic + 128],
                        wv_t[:, dc, :],
                        start=(dc == 0),
                        stop=(dc == 5),
                    )
                for h in range(HPC):
                    nc.vector.tensor_copy(
                        out=vaug[:, ic, h, 0:64],
                        in_=ps[:, 64 * h : 64 * h + 64],
                    )

            # ---- attention per head ----
            for h in range(HPC):
                if h < 2:
                    cc, pb = 0, 64 * h
                else:
                    cc, pb = 1, 0
                strips = []
                for jc in range(NJ):
                    i0 = 128 * jc
                    W = N - i0
                    strip = work.tile(
                        [P, W], f16, tag=f"expT{jc}", bufs=1, name=f"expT{jc}"
                    )
                    for s0 in range(0, W, 1024):
                        sw = min(1024, W - s0)
                        ps = psum_big.tile([P, 1024], f32, tag="big", name="ps_s")
                        for sub in range(0, sw, 512):
                            ssw = min(512, sw - sub)
                            nc.tensor.matmul(
                                ps[:, sub : sub + ssw],
                                kT[pb : pb + 64, cc, i0 : i0 + 128],
                                qT[pb : pb + 64, cc, i0 + s0 + sub : i0 + s0 + sub + ssw],
                                start=True,
                                stop=True,
                            )
                        nc.scalar.activation(
                            strip[:, s0 : s0 + sw],
                            ps[:, 0:sw],
                            EXP,
                            scale=0.125,
                        )
                    # causal mask on the diagonal 128-block
                    nc.vector.tensor_tensor(
                        strip[:, 0:128], strip[:, 0:128], mask_t[:], MULT
                    )
                    strips.append(strip)
                for ic in range(NI):
                    pa = psum_sa.tile([P, 128], f32, tag="sa", name="pa")
                    for jc in range(ic + 1):
                        off = 128 * (ic - jc)
                        nc.tensor.matmul(
                            pa[:, 0:65],
                            strips[jc][:, off : off + 128],
                            vaug[:, jc, h, :],
                            start=(jc == 0),
                            stop=(jc == ic),
                        )
                    recip = work.tile([P, 1], f32, tag="recip", name="recip")
                    nc.vector.reciprocal(recip[:], pa[:, 64:65])
                    sa_n = work.tile([P, 64], f16, tag="sa_n", name="sa_n")
                    nc.vector.tensor_scalar(
                        sa_n[:], pa[:, 0:64], recip[:, 0:1], None, op0=MULT
                    )
                    # transpose into the head's partition slot of saT
                    pt = psum_tr.tile([P, 128], f16, tag="tr", name="pt")
                    dst_chunk, dst_lo = (0, 64 * h) if h < 2 else (1, 0)
                    nc.tensor.transpose(
                        pt[dst_lo : dst_lo + 64, :], sa_n[:], ident[:]
                    )
                    nc.vector.tensor_copy(
                        out=saT[dst_lo : dst_lo + 64, dst_chunk,
                                128 * ic : 128 * ic + 128],
                        in_=pt[dst_lo : dst_lo + 64, :],
                    )

            # ---- GEMM2: yT = wp_pair^T @ saT_pair + wp_h2^T @ saT_h2 ----
            for oc in range(6):
                for isl in range(4):
                    ps = psum_big.tile([P, 1024], f32, tag="big", name="ps_y")
                    nc.tensor.matmul(
                        ps[:, 0:512],
                        wp_t[:, 0, 128 * oc : 128 * oc + 128],
                        saT[:, 0, 512 * isl : 512 * isl + 512],
                        start=True,
                        stop=False,
                    )
                    nc.tensor.matmul(
                        ps[:, 0:512],
                        wp_t[0:64, 1, 128 * oc : 128 * oc + 128],
                        saT[0:64, 1, 512 * isl : 512 * isl + 512],
                        start=False,
                        stop=True,
                    )
                    yst = ypool.tile([P, 512], f32, tag="yst", name="yst")
                    nc.vector.tensor_copy(out=yst[:], in_=ps[:, 0:512])
                    nc.sync.dma_start(
                        yT_v[:, oc, 512 * isl : 512 * isl + 512], yst[:]
                    )

    nc.compile()
    return nc


def _host_prep(x, Wkqv, bkqv, Wproj, bproj):
    f16 = np.float16
    Wk = Wkqv[:, 0:D]
    Wq = Wkqv[:, D : 2 * D]
    Wv = Wkqv[:, 2 * D : 3 * D]
    bk = bkqv[0:D]
    bq = bkqv[D : 2 * D]
    bv = bkqv[2 * D : 3 * D]
    out_bias = (bproj + bv @ Wproj).astype(np.float32)  # softmax rows sum to 1

    jmask = np.triu(np.ones((P, P), f16))  # mask[j,i] = 1 if i >= j
    ident = np.eye(P, dtype=f16)

    in_maps = []
    for b in range(B):
        xT = np.ascontiguousarray(x[b].T.astype(f16))
        for g in range(NG):
            hs = [HPC * g + i for i in range(HPC)]
            wk = [np.asarray(Wk[:, HD * h : HD * h + HD]) for h in hs]
            wq = [np.asarray(Wq[:, HD * h : HD * h + HD]) for h in hs]
            wv = [np.asarray(Wv[:, HD * h : HD * h + HD]) for h in hs]
            wkq = np.concatenate(
                [wk[0], wk[1], wq[0], wq[1], wk[2], wq[2]], axis=1
            ).astype(f16)
            wv_c = np.concatenate(wv, axis=1).astype(f16)
            # wp: chunk 0 = [h0 rows | h1 rows] (128), chunk 1 = [h2 rows | 0]
            wp = np.zeros((P, 2, D), f16)
            wp[0:64, 0] = Wproj[HD * hs[0] : HD * hs[0] + HD, :]
            wp[64:128, 0] = Wproj[HD * hs[1] : HD * hs[1] + HD, :]
            wp[0:64, 1] = Wproj[HD * hs[2] : HD * hs[2] + HD, :]
            bkq = np.zeros((P, 4), np.float32)
            bkq[:, 0] = np.concatenate(
                [bk[HD * hs[0] : HD * hs[0] + HD], bk[HD * hs[1] : HD * hs[1] + HD]]
            )
            bkq[:, 1] = np.concatenate(
                [bq[HD * hs[0] : HD * hs[0] + HD], bq[HD * hs[1] : HD * hs[1] + HD]]
            )
            bkq[0:64, 2] = bk[HD * hs[2] : HD * hs[2] + HD]
            bkq[0:64, 3] = bq[HD * hs[2] : HD * hs[2] + HD]
            in_maps.append(
                dict(xT=xT, wkq=wkq, wv=wv_c, wp=wp, bkq=bkq,
                     mask=jmask, ident=ident)
            )
    return in_maps, out_bias


def kernel(x, Wkqv, bkqv, Wproj, bproj):
    global _compiled, last_exec_time_ns, last_results
    import concourse.bass_utils as bass_utils

    x = np.asarray(x, np.float32)
    Wkqv = np.asarray(Wkqv, np.float32)
    bkqv = np.asarray(bkqv, np.float32)
    Wproj = np.asarray(Wproj, np.float32)
    bproj = np.asarray(bproj, np.float32)

    if _compiled is None:
        _compiled = _build()
    nc = _compiled

    in_maps, out_bias = _host_prep(x, Wkqv, bkqv, Wproj, bproj)

    trace = os.environ.get("BASS_KERNEL_TRACE", "0") == "1"
    res = bass_utils.run_bass_kernel_spmd(
        nc, in_maps, core_ids=list(range(NCORES)), trace=trace
    )
    last_exec_time_ns = res.exec_time_ns
    last_results = res

    out = np.zeros((B, N, D), np.float32)
    for b in range(B):
        acc = np.zeros((D, N), np.float32)
        for g in range(NG):
            acc += res.results[b * NG + g]["yT"]
        out[b] = acc.T + out_bias
    return out
